# revision 10
# baseline (speedup 1.0000x reference)
"""Causal self-attention (B=2, T=2048, C=1024, H=16) on 8 trn2 NeuronCores.

Sharding: 16 heads / 8 cores = 2 heads per core (both batches on every core).
Per core, for its head pair (h0 at partitions 0-63, h1 at 64-127):
  - QKV projection of the full sequence (384 weight columns), producing
    qT/kT in [head_dim, T] layout and V' in [T, head_dim] layout via PE
    transposes, with a ones column appended per head (softmax denominator).
  - Flash-style causal attention on-chip: the two heads' S^T tiles are
    computed as concurrent row-group matmuls into one 2-bank PSUM tile,
    one ACT exp covers both heads, causal masking multiplies a triangular
    0/1 mask on diagonal tiles only (GpSimd), att@V accumulates per head
    with the ones column yielding the denominator row.
  - Output projection as concurrent row-group matmul pairs (y_h1 shifted
    to partitions 64-127); softmax normalization is applied on the
    PSUM->SBUF copies as a per-partition (per-query) reciprocal scale,
    transported via a DRAM-bounce transpose of the denominator rows.
Host glue: transpose/round x, slice weights per core, sum the 8 partial
outputs, add b_proj.

Matmuls run in float32r (fp32 with 12-bit mantissa, 4x faster than fp32 on
the PE, full fp32 PSUM accumulate). Operands are pre-rounded on host or
rounded by the producing engine (f32r output APs).
"""

import sys

sys.path.insert(0, "/opt/trn_rl_repo")

import numpy as np

B, T, C, H, HD = 2, 2048, 1024, 16, 64
BT = B * T
NCORE = 8
HPC = H // NCORE  # heads per core
NT = BT // 512    # T-tiles for qkv projection
CCH = C // 128    # contraction chunks


def _round_f32r(x):
    x = np.ascontiguousarray(x, dtype=np.float32)
    xi = x.view(np.uint32)
    r = (xi + np.uint32(0x7FF) + ((xi >> np.uint32(12)) & np.uint32(1))) & np.uint32(
        0xFFFFF000
    )
    return r.view(np.float32)


_CACHE = {}


def _build():
    if "nc" in _CACHE:
        return _CACHE["nc"]
    from contextlib import ExitStack

    import concourse.bacc as bacc
    import concourse.mybir as mybir
    import concourse.tile as tile
    from concourse.masks import make_identity, make_upper_triangular

    f32, f32r = mybir.dt.float32, mybir.dt.float32r
    AF = mybir.ActivationFunctionType
    ALU = mybir.AluOpType

    nc = bacc.Bacc(None, target_bir_lowering=False, debug=False)
    xT_d = nc.dram_tensor("xT", [C, BT], f32r, kind="ExternalInput")
    wqkv_d = nc.dram_tensor("wqkv", [128, CCH, 3 * 128], f32r, kind="ExternalInput")
    bqkv_d = nc.dram_tensor("bqkv", [128, 3], f32, kind="ExternalInput")
    wp_d = nc.dram_tensor("wp", [128, C], f32r, kind="ExternalInput")
    out_d = nc.dram_tensor("out", [BT, C], f32, kind="ExternalOutput")

    with tile.TileContext(nc) as tc, ExitStack() as ctx:
        sb = ctx.enter_context(tc.tile_pool(name="sb", bufs=1))
        xp = ctx.enter_context(tc.tile_pool(name="xp", bufs=3))
        vtp = ctx.enter_context(tc.tile_pool(name="vtp", bufs=2))
        esp = ctx.enter_context(tc.tile_pool(name="esp", bufs=3))
        ytp = ctx.enter_context(tc.tile_pool(name="ytp", bufs=3))
        dnp = ctx.enter_context(tc.tile_pool(name="dnp", bufs=4))
        outp = ctx.enter_context(tc.tile_pool(name="outp", bufs=3))
        drp = ctx.enter_context(tc.tile_pool(name="drp", bufs=8, space="DRAM"))
        # PSUM: 2 + 4 + 2 = 8 banks
        pa = ctx.enter_context(tc.tile_pool(name="pa", bufs=2, space="PSUM"))
        pss = ctx.enter_context(tc.tile_pool(name="pss", bufs=2, space="PSUM"))
        pso = ctx.enter_context(tc.tile_pool(name="pso", bufs=2, space="PSUM"))

        wq_sb = sb.tile([128, CCH, 3 * 128], f32r, tag="wq")

        qT = [sb.tile([128, T], f32r, tag=f"qT{b}", name=f"qT{b}") for b in range(B)]
        kT = [sb.tile([128, T], f32r, tag=f"kT{b}", name=f"kT{b}") for b in range(B)]
        # V' per batch: per k-tile [V_h0 (64) | 1 | V_h1 (64) | 1 | pad] = 196
        # cols. The att@v lhsT slices are padded to 128 columns ([0:128] and
        # [65:193]) so every matmul lights up all 16 PE sub-arrays — at 65
        # columns the HAM activity monitor reads the PE as half-idle and
        # clamps the clock to 1.2 GHz. Output rows past 64 are junk, unread.
        VW = 196
        Vp = [
            sb.tile([128, 16, VW], f32r, tag=f"Vp{b}", name=f"Vp{b}") for b in range(B)
        ]
        for b in range(B):
            # ones columns (64, 129) and finite pad from blanket 1.0 fill;
            # V columns are overwritten by the transpose copies below
            nc.vector.memset(Vp[b][:, :, :].bitcast(f32), 1.0)

        # ---------------- Phase A: QKV projection ----------------
        xT_r = xT_d.rearrange("(cc p) t -> p cc t", p=128)
        x_tiles = []
        for tt in range(NT):
            x_t = xp.tile([128, CCH, 512], f32r, tag="x", name=f"x{tt}")
            if tt == 0:
                # chunked loads so the first matmul chain starts after ~400KB
                # instead of after the whole 3.5MB of weights+x
                for cc in range(CCH):
                    nc.sync.dma_start(
                        out=wq_sb[:, cc, :], in_=wqkv_d[:, cc, :]
                    )
                    nc.sync.dma_start(
                        out=x_t[:, cc, :], in_=xT_r[:, cc, 0:512]
                    )
            else:
                nc.sync.dma_start(out=x_t, in_=xT_r[:, :, tt * 512 : (tt + 1) * 512])
            x_tiles.append(x_t)
            if tt == 0:
                # constants that are not needed until later: emit their loads
                # after the first x tile so the first matmul starts sooner
                bias_sb = sb.tile([128, 3], f32, tag="bias")
                nc.sync.dma_start(out=bias_sb, in_=bqkv_d[:, :])
                wp_sb = sb.tile([128, C], f32r, tag="wp")
                nc.sync.dma_start(out=wp_sb, in_=wp_d[:, :])
                ident = sb.tile([128, 128], f32, tag="ident")
                make_identity(nc, ident)
                tri2 = sb.tile([128, 2, 128], f32, tag="tri2")
                make_upper_triangular(nc, tri2[:, 0, :], val=1.0, diag=True)
                nc.gpsimd.tensor_copy(tri2[:, 1, :], tri2[:, 0, :])

        for tt in range(NT):
            b = tt // (NT // B)
            tloc = (tt % (NT // B)) * 512
            x_t = x_tiles[tt]
            for g in range(3):
                ps = pa.tile([128, 512], f32, tag="mm")
                for cc in range(CCH):
                    nc.tensor.matmul(
                        ps,
                        wq_sb[:, cc, g * 128 : (g + 1) * 128],
                        x_t[:, cc, :],
                        start=(cc == 0),
                        stop=(cc == CCH - 1),
                    )
                if g == 0:
                    nc.vector.tensor_scalar_add(
                        qT[b][:, tloc : tloc + 512], ps, bias_sb[:, 0:1]
                    )
                elif g == 1:
                    nc.vector.tensor_scalar_add(
                        kT[b][:, tloc : tloc + 512], ps, bias_sb[:, 1:2]
                    )
                else:
                    v_t = vtp.tile([128, 512], f32, tag="v")
                    nc.vector.tensor_scalar_add(v_t, ps, bias_sb[:, 2:3])
                    for j in range(4):
                        pt = pso.tile([128, 128], f32, tag="po")
                        nc.tensor.transpose(pt, v_t[:, j * 128 : (j + 1) * 128], ident)
                        ktl = (tt % (NT // B)) * 4 + j
                        # one strided copy moves both heads' V columns
                        nc.vector.tensor_copy(
                            Vp[b][:, ktl, 0:130].rearrange("p (s e) -> p s e", s=2)[
                                :, :, 0:64
                            ],
                            pt[:, :].rearrange("p (s e) -> p s e", s=2),
                        )

        # ------------- Phase B: attention + output projection -------------
        scale = 1.0 / 8.0  # 1/sqrt(HD)
        for b in range(B):
            for qb in range(4):
                n_kt = 4 * (qb + 1)
                po = [
                    pso.tile([128, 512], f32, tag="po", name=f"po{b}{qb}{h}")
                    for h in range(2)
                ]
                pend = []  # att@v pipelined one k-tile behind S/exp
                for lkt in range(n_kt):
                    r0 = max(0, (lkt - 4 * qb) * 128)
                    ks = slice(lkt * 128, (lkt + 1) * 128)
                    qs = slice(qb * 512 + r0, (qb + 1) * 512)
                    ps2 = pss.tile([128, 1024], f32, tag="s2")
                    nc.tensor.matmul(
                        ps2[:, r0:512], kT[b][0:64, ks], qT[b][0:64, qs],
                        start=True, stop=True,
                    )
                    nc.tensor.matmul(
                        ps2[:, 512 + r0 : 1024], kT[b][64:128, ks], qT[b][64:128, qs],
                        start=True, stop=True,
                    )
                    es = esp.tile([128, 1024], f32r, tag="es")
                    if r0:
                        nc.scalar.activation(
                            es[:, :].rearrange("p (h q) -> p h q", h=2)[:, :, r0:512],
                            ps2[:, :].rearrange("p (h q) -> p h q", h=2)[:, :, r0:512],
                            AF.Exp,
                            scale=scale,
                        )
                    else:
                        nc.scalar.activation(es, ps2, AF.Exp, scale=scale)
                    if lkt >= 4 * qb:  # diagonal tile: causal mask, both heads
                        nc.gpsimd.tensor_mul(
                            es[:, :].rearrange("p (h q) -> p h q", h=2)[
                                :, :, r0 : r0 + 128
                            ],
                            es[:, :].rearrange("p (h q) -> p h q", h=2)[
                                :, :, r0 : r0 + 128
                            ],
                            tri2[:, :, :],
                        )
                    for mm in pend:
                        nc.tensor.matmul(**mm)
                    pend = [
                        dict(
                            out=po[h][:, r0:512],
                            lhsT=Vp[b][:, lkt, h * 65 : h * 65 + 128],
                            rhs=es[:, h * 512 + r0 : (h + 1) * 512],
                            start=(lkt == 0),
                            stop=(lkt == n_kt - 1),
                        )
                        for h in range(2)
                    ]
                for mm in pend:
                    nc.tensor.matmul(**mm)

                # pack y (h1 shifted to partitions 64-127), collect denominators
                ytb = ytp.tile([128, 512], f32r, tag="ytb")
                nc.vector.tensor_copy(ytb[0:64, :], po[0][0:64, :])
                nc.vector.tensor_copy(ytb[64:128, :], po[1][0:64, :])
                dh = ytp.tile([65, 1024], f32, tag="dh")
                nc.vector.tensor_copy(dh[64:65, 0:512], po[0][64:65, :])
                nc.vector.tensor_copy(dh[64:65, 512:1024], po[1][64:65, :])
                scr = drp.tile([1, 1024], f32, tag="scr")
                nc.gpsimd.dma_start(out=scr[0:1, :], in_=dh[64:65, :])
                dn = dnp.tile([128, 8], f32, tag="dn")
                nc.gpsimd.dma_start(
                    out=dn, in_=scr[0, :].rearrange("(m p) -> p m", p=128)
                )
                rc = dnp.tile([128, 8], f32, tag="rc")
                nc.vector.reciprocal(rc, dn)

                # output projection: concurrent row-group pair per (j, ncol)
                for j in range(4):
                    out_t = outp.tile([128, C], f32, tag="out")
                    js = slice(j * 128, (j + 1) * 128)
                    for ncol in range(2):
                        cs = slice(ncol * 512, (ncol + 1) * 512)
                        pp0 = pa.tile([128, 512], f32, tag="mm", name="pp0")
                        nc.tensor.matmul(
                            pp0, ytb[0:64, js], wp_sb[0:64, cs], start=True, stop=True
                        )
                        pp1 = pa.tile([128, 512], f32, tag="mm", name="pp1")
                        nc.tensor.matmul(
                            pp1, ytb[64:128, js], wp_sb[64:128, cs],
                            start=True, stop=True,
                        )
                        nc.scalar.activation(
                            out_t[:, cs], pp0, AF.Copy, scale=rc[:, j : j + 1]
                        )
                        nc.vector.scalar_tensor_tensor(
                            out_t[:, cs],
                            pp1,
                            rc[:, 4 + j : 5 + j],
                            out_t[:, cs],
                            ALU.mult,
                            ALU.add,
                        )
                    row = b * T + qb * 512 + j * 128
                    nc.sync.dma_start(out=out_d[row : row + 128, :], in_=out_t)

    nc.finalize()
    _CACHE["nc"] = nc
    return nc


def _prep_inputs(x, w_attn, b_attn, w_proj):
    x = np.ascontiguousarray(np.asarray(x, dtype=np.float32))
    w_attn = np.asarray(w_attn, dtype=np.float32)
    b_attn = np.asarray(b_attn, dtype=np.float32)
    w_proj = np.asarray(w_proj, dtype=np.float32)

    xT = _round_f32r(x.reshape(BT, C).T)
    in_maps = []
    for c in range(NCORE):
        hs = [HPC * c + j for j in range(HPC)]
        blocks = []
        bias_cols = []
        for off in (0, C, 2 * C):
            for h in hs:
                blocks.append(w_attn[:, off + h * HD : off + (h + 1) * HD])
            bias_cols.append(
                np.concatenate([b_attn[off + h * HD : off + (h + 1) * HD] for h in hs])
            )
        wq_flat = _round_f32r(np.concatenate(blocks, axis=1))  # [C, 384]
        wqkv = np.ascontiguousarray(wq_flat.reshape(CCH, 128, 3 * 128).transpose(1, 0, 2))
        bqkv = np.ascontiguousarray(np.stack(bias_cols, axis=1))  # [128, 3]
        wp = _round_f32r(
            np.concatenate([w_proj[h * HD : (h + 1) * HD, :] for h in hs], axis=0)
        )  # [128, C]
        in_maps.append({"xT": xT, "wqkv": wqkv, "bqkv": bqkv, "wp": wp})
    return in_maps


def _run(x, w_attn, b_attn, w_proj, b_proj, trace=False, tmpdir=None):
    from concourse.bass_utils import run_bass_kernel_spmd

    nc = _build()
    in_maps = _prep_inputs(x, w_attn, b_attn, w_proj)
    res = run_bass_kernel_spmd(
        nc, in_maps, list(range(NCORE)), trace=trace, tmpdir=tmpdir
    )
    acc = np.sum(
        np.stack([res.results[i]["out"] for i in range(NCORE)]), axis=0, dtype=np.float64
    )
    out = (acc + np.asarray(b_proj, dtype=np.float64)).astype(np.float32)
    return out.reshape(B, T, C), res


def kernel(x, w_attn, b_attn, w_proj, b_proj):
    out, _ = _run(x, w_attn, b_attn, w_proj, b_proj, trace=False)
    return out
